# revision 6
# baseline (speedup 1.0000x reference)
"""Causal self-attention (B=1, T=4096, E=1024, H=16, D=64) on 8 TRN2 NeuronCores.

Sharding: tensor-parallel over heads — each core owns 2 heads (128 of the
1024 hidden dims). Each core computes its slice of the QKV projection, a
flash-style causal attention for its 2 heads, and a partial output
projection (rows of w_out for its head dims). The host sums the 8 partial
outputs (the row-parallel all-reduce) and adds b_out.

Matmul inputs are bf16 (1 cycle/row on the PE, vs 4 for fp32), all
accumulation is fp32 in PSUM. x and the weights are converted to bf16 on
the host, which also halves their DMA traffic.

Per-core dataflow (feature-major / transposed throughout, to avoid
transposing x or P):
  qT/kT/vT [128, 4096]  = w_slice.T @ x.T      (K=e chunks of 128)
  V' [tk, 65] per head  = PE-transpose of vT + ones column
  per (head, 512-wide tq block):
    for each 128-tk block:  S^T = kT_blk.T @ qT_blk   [tk=128, tq=512] PSUM
                            P = exp(0.125 * S^T)      ACT, PSUM->SBUF bf16
                            (causal: affine_select zeroes tk > tq)
                            O' += V'_blk.T @ P        [65, tq=512] PSUM accum
    row 64 of O' = softmax denominators (ones column trick).
    normalize via PE transpose + reciprocal + per-partition scale,
    transpose back -> UnT [hd=128, t] (both heads stacked)
  out_partial[t, :] = UnT_tile.T @ w_out_rows   (K=128)
"""

import sys

for _p in ("/opt/trn_rl_repo",):
    if _p not in sys.path:
        sys.path.insert(0, _p)

import ml_dtypes
import numpy as np

import concourse.bass as bass  # noqa: F401
import concourse.mybir as mybir
import concourse.tile as tile
from concourse import bacc
from concourse.bass_utils import run_bass_kernel_spmd
from concourse.masks import make_identity

T, E = 4096, 1024
H, D = 16, 64
NCORES = 8
HPC = H // NCORES          # heads per core = 2
HD = HPC * D               # hidden dims per core = 128
NT = T // 512              # 8 t-chunks of 512
NE = E // 128              # 8 e-chunks of 128
NTB = T // 128             # 32 tk blocks of 128

F32 = mybir.dt.float32
BF16 = mybir.dt.bfloat16
NPBF16 = np.dtype(ml_dtypes.bfloat16)
AF = mybir.ActivationFunctionType


def _build_kernel():
    nc = bacc.Bacc("TRN2", target_bir_lowering=False, debug=False)

    xT = nc.dram_tensor("xT", [E, T], BF16, kind="ExternalInput")
    wq = nc.dram_tensor("wq", [E, HD], BF16, kind="ExternalInput")
    wk = nc.dram_tensor("wk", [E, HD], BF16, kind="ExternalInput")
    wv = nc.dram_tensor("wv", [E, HD], BF16, kind="ExternalInput")
    bqkv = nc.dram_tensor("bqkv", [3, HD, 1], F32, kind="ExternalInput")
    wo = nc.dram_tensor("wo", [HD, E], BF16, kind="ExternalInput")
    out = nc.dram_tensor("out", [T, E], F32, kind="ExternalOutput")

    with tile.TileContext(nc) as tc:
        _body(nc, tc, xT, wq, wk, wv, bqkv, wo, out)
    nc.compile()
    return nc


def _body(nc, tc, xT, wq, wk, wv, bqkv, wo, out):
    from contextlib import ExitStack

    ctx = ExitStack()
    with ctx:
        const = ctx.enter_context(tc.tile_pool(name="const", bufs=1))
        big = ctx.enter_context(tc.tile_pool(name="big", bufs=1))
        xpool = ctx.enter_context(tc.tile_pool(name="xp", bufs=16))
        ppool = ctx.enter_context(tc.tile_pool(name="pp", bufs=3))
        opool = ctx.enter_context(tc.tile_pool(name="op", bufs=3))
        small = ctx.enter_context(tc.tile_pool(name="sm", bufs=4))
        ps_mm = ctx.enter_context(tc.tile_pool(name="ps_mm", bufs=2, space="PSUM"))
        ps_o = ctx.enter_context(tc.tile_pool(name="ps_o", bufs=2, space="PSUM"))
        ps_t2 = ctx.enter_context(tc.tile_pool(name="ps_t2", bufs=2, space="PSUM"))

        # ---- constants / weights ----
        ident = const.tile([128, 128], F32)
        make_identity(nc, ident[:])
        identb = const.tile([128, 128], BF16)
        make_identity(nc, identb[:])

        wq_sb = const.tile([128, NE, HD], BF16)
        wk_sb = const.tile([128, NE, HD], BF16)
        wv_sb = const.tile([128, NE, HD], BF16)
        for w_dram, w_sb in ((wq, wq_sb), (wk, wk_sb), (wv, wv_sb)):
            nc.sync.dma_start(
                w_sb[:], w_dram[:].rearrange("(a p) c -> p a c", p=128)
            )
        wo_sb = const.tile([128, E], BF16)
        nc.sync.dma_start(wo_sb[:], wo[:])

        bq_sb = const.tile([128, 1], F32)
        bk_sb = const.tile([128, 1], F32)
        bv_sb = const.tile([128, 1], F32)
        nc.sync.dma_start(bq_sb[:], bqkv[0])
        nc.sync.dma_start(bk_sb[:], bqkv[1])
        nc.sync.dma_start(bv_sb[:], bqkv[2])

        qT = big.tile([128, T], BF16)
        kT = big.tile([128, T], BF16)
        vT = big.tile([128, T], BF16)
        # V row-major, per head with a ones column: [:, tb, h*65 + 0:64] is
        # V_h for tk block tb, [:, tb, h*65 + 64] is the ones column.
        V2 = big.tile([128, NTB, 2 * (D + 1)], BF16)
        # normalized attention outputs, transposed: rows h*64+d, cols t
        UnT = big.tile([128, T], BF16)

        nc.gpsimd.memset(V2[:, :, D], 1.0)
        nc.gpsimd.memset(V2[:, :, D + 1 + D], 1.0)

        # ---- phase A: QKV^T projection (+ immediate V' transposes) ----
        for tcc in range(NT):
            xs = []
            for ec in range(NE):
                xsb = xpool.tile([128, 512], BF16, tag="xsb")
                nc.sync.dma_start(
                    xsb[:], xT[ec * 128:(ec + 1) * 128, tcc * 512:(tcc + 1) * 512]
                )
                xs.append(xsb)
            for w_sb, b_sb, dst in ((wq_sb, bq_sb, qT), (wk_sb, bk_sb, kT),
                                    (wv_sb, bv_sb, vT)):
                ps = ps_mm.tile([128, 512], F32, tag="mm")
                for ec in range(NE):
                    nc.tensor.matmul(
                        ps[:], w_sb[:, ec, :], xs[ec][:],
                        start=(ec == 0), stop=(ec == NE - 1),
                    )
                nc.vector.tensor_scalar_add(
                    dst[:, tcc * 512:(tcc + 1) * 512], ps[:], b_sb[:]
                )
            # build row-major V' for this chunk right away (keeps PE duty
            # high — a long stretch of short transposes at the phase
            # boundary re-throttles the PE clock)
            for tb in range(4 * tcc, 4 * (tcc + 1)):
                for h in range(HPC):
                    pst = ps_t2.tile([128, 128], BF16, tag="t2")
                    nc.tensor.transpose(
                        pst[:, 0:D],
                        vT[h * D:(h + 1) * D, tb * 128:(tb + 1) * 128],
                        identb[h * D:(h + 1) * D, h * D:(h + 1) * D],
                    )
                    nc.vector.tensor_copy(
                        V2[:, tb, h * (D + 1):h * (D + 1) + D], pst[:, 0:D]
                    )

        # ---- phase B+C: attention per 512-wide tq block, then out-proj ----
        for qb in range(NT):
            nblk = 4 * (qb + 1)
            for h in range(HPC):
                hs = slice(h * D, (h + 1) * D)
                po = ps_o.tile([D + 1, 512], F32, tag="o")
                for p in range(nblk // 2):
                    tb0 = 2 * p
                    diag = tb0 + 1 >= 4 * qb  # pair touches the diagonal
                    psS = ps_mm.tile([128, 1024], F32, tag="mm")
                    P = ppool.tile([128, 1024], BF16, tag="P")
                    if not diag:
                        for s in range(2):
                            tb = tb0 + s
                            nc.tensor.matmul(
                                psS[:, s * 512:(s + 1) * 512],
                                kT[hs, tb * 128:(tb + 1) * 128],
                                qT[hs, qb * 512:(qb + 1) * 512],
                                start=True, stop=True,
                            )
                        nc.scalar.activation(P[:], psS[:], AF.Exp, scale=0.125)
                        for s in range(2):
                            tb = tb0 + s
                            nc.tensor.matmul(
                                po[:],
                                V2[:, tb, h * (D + 1):(h + 1) * (D + 1)],
                                P[:, s * 512:(s + 1) * 512],
                                start=(tb == 0), stop=(tb == nblk - 1),
                            )
                    else:
                        j0 = tb0 - 4 * qb  # 0 or 2
                        for s in range(2):
                            tb = tb0 + s
                            jj = j0 + s
                            nc.tensor.matmul(
                                psS[:, s * 512 + jj * 128:(s + 1) * 512],
                                kT[hs, tb * 128:(tb + 1) * 128],
                                qT[hs, qb * 512 + jj * 128:(qb + 1) * 512],
                                start=True, stop=True,
                            )
                        nc.scalar.activation(
                            P[:, j0 * 128:], psS[:, j0 * 128:],
                            AF.Exp, scale=0.125,
                        )
                        # keep where tq >= tk over the [128, 2, 512] view:
                        # iota = (qb*512 - tb0*128) - p - 128*s + f
                        nc.gpsimd.affine_select(
                            out=P[:].rearrange("p (a b) -> p a b", a=2),
                            in_=P[:].rearrange("p (a b) -> p a b", a=2),
                            compare_op=mybir.AluOpType.is_ge,
                            fill=0.0,
                            base=qb * 512 - tb0 * 128,
                            channel_multiplier=-1,
                            pattern=[[-128, 2], [1, 512]],
                        )
                        for s in range(2):
                            tb = tb0 + s
                            jj = j0 + s
                            nc.tensor.matmul(
                                po[:, jj * 128:512],
                                V2[:, tb, h * (D + 1):(h + 1) * (D + 1)],
                                P[:, s * 512 + jj * 128:(s + 1) * 512],
                                start=(tb == 0), stop=(tb == nblk - 1),
                            )
                # normalize: U = O'[0:64] * (1 / O'[64]) columnwise
                rec1 = small.tile([1, 512], F32, tag="rec1")
                nc.vector.reciprocal(rec1[:], po[D:D + 1, :])
                rb = small.tile([D, 512], F32, tag="rb")
                nc.gpsimd.partition_broadcast(rb[:], rec1[:], channels=D)
                nc.vector.tensor_mul(
                    UnT[hs, qb * 512:(qb + 1) * 512], po[0:D, :], rb[:]
                )
            # out-proj for the 4 t-tiles of this q block
            for tt in range(qb * 4, (qb + 1) * 4):
                osb2 = opool.tile([128, E], F32, tag="out")
                for half in range(2):
                    psc = ps_mm.tile([128, 512], F32, tag="mm")
                    nc.tensor.matmul(
                        psc[:],
                        UnT[:, tt * 128:(tt + 1) * 128],
                        wo_sb[:, half * 512:(half + 1) * 512],
                        start=True, stop=True,
                    )
                    nc.vector.tensor_copy(
                        osb2[:, half * 512:(half + 1) * 512], psc[:]
                    )
                nc.sync.dma_start(out[tt * 128:(tt + 1) * 128, :], osb2[:])


_NC_CACHE = None


def _get_nc():
    global _NC_CACHE
    if _NC_CACHE is None:
        _NC_CACHE = _build_kernel()
    return _NC_CACHE


def _make_in_maps(x, w_qkv, b_qkv, w_out):
    x2 = np.asarray(x, dtype=np.float32).reshape(T, E)
    xT = np.ascontiguousarray(x2.T).astype(NPBF16)
    w_qkv = np.asarray(w_qkv, dtype=np.float32)
    b_qkv = np.asarray(b_qkv, dtype=np.float32)
    w_out = np.asarray(w_out, dtype=np.float32)
    in_maps = []
    for c in range(NCORES):
        s = slice(c * HD, (c + 1) * HD)
        in_maps.append({
            "xT": xT,
            "wq": np.ascontiguousarray(
                w_qkv[:, 0 * E + c * HD:0 * E + (c + 1) * HD]).astype(NPBF16),
            "wk": np.ascontiguousarray(
                w_qkv[:, 1 * E + c * HD:1 * E + (c + 1) * HD]).astype(NPBF16),
            "wv": np.ascontiguousarray(
                w_qkv[:, 2 * E + c * HD:2 * E + (c + 1) * HD]).astype(NPBF16),
            "bqkv": np.ascontiguousarray(
                np.stack([
                    b_qkv[0 * E + c * HD:0 * E + (c + 1) * HD],
                    b_qkv[1 * E + c * HD:1 * E + (c + 1) * HD],
                    b_qkv[2 * E + c * HD:2 * E + (c + 1) * HD],
                ]).reshape(3, HD, 1)
            ),
            "wo": np.ascontiguousarray(w_out[s, :]).astype(NPBF16),
        })
    return in_maps


def run_sharded(x, w_qkv, b_qkv, w_out, b_out, trace=False):
    """Run the SPMD kernel; returns (full_output, BassKernelResults)."""
    nc = _get_nc()
    in_maps = _make_in_maps(x, w_qkv, b_qkv, w_out)
    res = run_bass_kernel_spmd(
        nc, in_maps, core_ids=list(range(NCORES)), trace=trace
    )
    acc = np.zeros((T, E), dtype=np.float32)
    for c in range(NCORES):
        acc += res.results[c]["out"]
    acc += np.asarray(b_out, dtype=np.float32)[None, :]
    return acc.reshape(1, T, E), res


def kernel(x, w_qkv, b_qkv, w_out, b_out):
    out, _ = run_sharded(x, w_qkv, b_qkv, w_out, b_out, trace=False)
    return out


# revision 9
# speedup vs baseline: 1.2611x; 1.2611x over previous
"""Causal self-attention (B=1, T=4096, E=1024, H=16, D=64) on 8 TRN2 NeuronCores.

Sharding: tensor-parallel over heads — each core owns 2 heads (128 of the
1024 hidden dims). Each core computes its slice of the QKV projection, a
flash-style causal attention for its 2 heads, and a partial output
projection (rows of w_out for its head dims). The host sums the 8 partial
outputs (the row-parallel all-reduce) and adds b_out.

Matmul inputs are bf16 (1 cycle/row on the PE, vs 4 for fp32), all
accumulation is fp32 in PSUM. x and the weights are converted to bf16 on
the host, which also halves their DMA traffic.

Per-core dataflow (feature-major / transposed throughout, to avoid
transposing x or P):
  qT/kT/vT [128, 4096]  = w_slice.T @ x.T      (K=e chunks of 128)
  V' [tk, 65] per head  = PE-transpose of vT + ones column
  per (head, 512-wide tq block):
    for each 128-tk block:  S^T = kT_blk.T @ qT_blk   [tk=128, tq=512] PSUM
                            P = exp(0.125 * S^T)      ACT, PSUM->SBUF bf16
                            (causal: affine_select zeroes tk > tq)
                            O' += V'_blk.T @ P        [65, tq=512] PSUM accum
    row 64 of O' = softmax denominators (ones column trick).
    normalize via PE transpose + reciprocal + per-partition scale,
    transpose back -> UnT [hd=128, t] (both heads stacked)
  out_partial[t, :] = UnT_tile.T @ w_out_rows   (K=128)
"""

import sys

for _p in ("/opt/trn_rl_repo",):
    if _p not in sys.path:
        sys.path.insert(0, _p)

import ml_dtypes
import numpy as np

import concourse.bass as bass  # noqa: F401
import concourse.mybir as mybir
import concourse.tile as tile
from concourse import bacc
from concourse.bass_utils import run_bass_kernel_spmd
from concourse.masks import make_identity

T, E = 4096, 1024
H, D = 16, 64
NCORES = 8
HPC = H // NCORES          # heads per core = 2
HD = HPC * D               # hidden dims per core = 128
NT = T // 512              # 8 t-chunks of 512
NE = E // 128              # 8 e-chunks of 128
NTB = T // 128             # 32 tk blocks of 128

F32 = mybir.dt.float32
BF16 = mybir.dt.bfloat16
NPBF16 = np.dtype(ml_dtypes.bfloat16)
AF = mybir.ActivationFunctionType


def _build_kernel():
    nc = bacc.Bacc("TRN2", target_bir_lowering=False, debug=False)

    xT = nc.dram_tensor("xT", [E, T], BF16, kind="ExternalInput")
    wq = nc.dram_tensor("wq", [E, HD], BF16, kind="ExternalInput")
    wk = nc.dram_tensor("wk", [E, HD], BF16, kind="ExternalInput")
    wv = nc.dram_tensor("wv", [E, HD], BF16, kind="ExternalInput")
    bqkv = nc.dram_tensor("bqkv", [3, HD, 1], F32, kind="ExternalInput")
    wo = nc.dram_tensor("wo", [HD, E], BF16, kind="ExternalInput")
    out = nc.dram_tensor("out", [T, E], F32, kind="ExternalOutput")

    with tile.TileContext(nc) as tc:
        _body(nc, tc, xT, wq, wk, wv, bqkv, wo, out)
    nc.compile()
    return nc


def _body(nc, tc, xT, wq, wk, wv, bqkv, wo, out):
    from contextlib import ExitStack

    ctx = ExitStack()
    with ctx:
        const = ctx.enter_context(tc.tile_pool(name="const", bufs=1))
        big = ctx.enter_context(tc.tile_pool(name="big", bufs=1))
        xpool = ctx.enter_context(tc.tile_pool(name="xp", bufs=16))
        ppool = ctx.enter_context(tc.tile_pool(name="pp", bufs=3))
        opool = ctx.enter_context(tc.tile_pool(name="op", bufs=3))
        small = ctx.enter_context(tc.tile_pool(name="sm", bufs=4))
        ps_mm = ctx.enter_context(tc.tile_pool(name="ps_mm", bufs=3, space="PSUM"))
        ps_o = ctx.enter_context(tc.tile_pool(name="ps_o", bufs=2, space="PSUM"))

        # ---- constants / weights ----
        ident = const.tile([128, 128], F32)
        make_identity(nc, ident[:])
        identb = const.tile([128, 128], BF16)
        make_identity(nc, identb[:])

        wq_sb = const.tile([128, NE, HD], BF16)
        wk_sb = const.tile([128, NE, HD], BF16)
        wv_sb = const.tile([128, NE, HD], BF16)
        for w_dram, w_sb in ((wq, wq_sb), (wk, wk_sb), (wv, wv_sb)):
            nc.sync.dma_start(
                w_sb[:], w_dram[:].rearrange("(a p) c -> p a c", p=128)
            )
        wo_sb = const.tile([128, E], BF16)
        nc.sync.dma_start(wo_sb[:], wo[:])

        bq_sb = const.tile([128, 1], F32)
        bk_sb = const.tile([128, 1], F32)
        bv_sb = const.tile([128, 1], F32)
        nc.sync.dma_start(bq_sb[:], bqkv[0])
        nc.sync.dma_start(bk_sb[:], bqkv[1])
        nc.sync.dma_start(bv_sb[:], bqkv[2])

        qT = big.tile([128, T], BF16)
        kT = big.tile([128, T], BF16)
        vT = big.tile([128, T], BF16)
        # V row-major, per head with a ones column: [:, tb, h*65 + 0:64] is
        # V_h for tk block tb, [:, tb, h*65 + 64] is the ones column.
        V2 = big.tile([128, NTB, 2 * (D + 1)], BF16)
        # normalized attention outputs, transposed: rows h*64+d, cols t
        UnT = big.tile([128, T], BF16)

        nc.gpsimd.memset(V2[:, :, D], 1.0)
        nc.gpsimd.memset(V2[:, :, D + 1 + D], 1.0)

        # ---- phase A: QKV^T projection (+ immediate V' transposes) ----
        for tcc in range(NT):
            xs = []
            for ec in range(NE):
                xsb = xpool.tile([128, 512], BF16, tag="xsb")
                nc.sync.dma_start(
                    xsb[:], xT[ec * 128:(ec + 1) * 128, tcc * 512:(tcc + 1) * 512]
                )
                xs.append(xsb)
            for w_sb, b_sb, dst in ((wq_sb, bq_sb, qT), (wk_sb, bk_sb, kT),
                                    (wv_sb, bv_sb, vT)):
                ps = ps_mm.tile([128, 512], F32, tag="mm")
                for ec in range(NE):
                    nc.tensor.matmul(
                        ps[:], w_sb[:, ec, :], xs[ec][:],
                        start=(ec == 0), stop=(ec == NE - 1),
                    )
                nc.vector.tensor_scalar_add(
                    dst[:, tcc * 512:(tcc + 1) * 512], ps[:], b_sb[:]
                )
            # build row-major V' for this chunk right away (keeps PE duty
            # high — a long stretch of short transposes at the phase
            # boundary re-throttles the PE clock)
            for tb in range(4 * tcc, 4 * (tcc + 1)):
                for h in range(HPC):
                    pst = ps_o.tile([128, 128], BF16, tag="o")
                    nc.tensor.transpose(
                        pst[:, 0:D],
                        vT[h * D:(h + 1) * D, tb * 128:(tb + 1) * 128],
                        identb[h * D:(h + 1) * D, h * D:(h + 1) * D],
                    )
                    nc.vector.tensor_copy(
                        V2[:, tb, h * (D + 1):h * (D + 1) + D], pst[:, 0:D]
                    )

        # ---- phase B+C: attention per 512-wide tq block, then out-proj ----
        for qb in range(NT):
            nblk = 4 * (qb + 1)
            for h in range(HPC):
                hs = slice(h * D, (h + 1) * D)
                po = ps_o.tile([D + 1, 512], F32, tag="o")
                for p in range(nblk // 2):
                    tb0 = 2 * p
                    diag = tb0 + 1 >= 4 * qb  # pair touches the diagonal
                    psS = ps_mm.tile([128, 1024], F32, tag="mm")
                    P = ppool.tile([128, 1024], BF16, tag="P")
                    if not diag:
                        for s in range(2):
                            tb = tb0 + s
                            nc.tensor.matmul(
                                psS[:, s * 512:(s + 1) * 512],
                                kT[hs, tb * 128:(tb + 1) * 128],
                                qT[hs, qb * 512:(qb + 1) * 512],
                                start=True, stop=True,
                            )
                        nc.scalar.activation(P[:], psS[:], AF.Exp, scale=0.125)
                        for s in range(2):
                            tb = tb0 + s
                            nc.tensor.matmul(
                                po[:],
                                V2[:, tb, h * (D + 1):(h + 1) * (D + 1)],
                                P[:, s * 512:(s + 1) * 512],
                                start=(tb == 0), stop=(tb == nblk - 1),
                            )
                    else:
                        j0 = tb0 - 4 * qb  # 0 or 2
                        for s in range(2):
                            tb = tb0 + s
                            jj = j0 + s
                            nc.tensor.matmul(
                                psS[:, s * 512 + jj * 128:(s + 1) * 512],
                                kT[hs, tb * 128:(tb + 1) * 128],
                                qT[hs, qb * 512 + jj * 128:(qb + 1) * 512],
                                start=True, stop=True,
                            )
                        nc.scalar.activation(
                            P[:, j0 * 128:], psS[:, j0 * 128:],
                            AF.Exp, scale=0.125,
                        )
                        # keep where tq >= tk over the [128, 2, 512] view:
                        # iota = (qb*512 - tb0*128) - p - 128*s + f
                        nc.gpsimd.affine_select(
                            out=P[:].rearrange("p (a b) -> p a b", a=2),
                            in_=P[:].rearrange("p (a b) -> p a b", a=2),
                            compare_op=mybir.AluOpType.is_ge,
                            fill=0.0,
                            base=qb * 512 - tb0 * 128,
                            channel_multiplier=-1,
                            pattern=[[-128, 2], [1, 512]],
                        )
                        for s in range(2):
                            tb = tb0 + s
                            jj = j0 + s
                            nc.tensor.matmul(
                                po[:, jj * 128:512],
                                V2[:, tb, h * (D + 1):(h + 1) * (D + 1)],
                                P[:, s * 512 + jj * 128:(s + 1) * 512],
                                start=(tb == 0), stop=(tb == nblk - 1),
                            )
                # normalize: U = O'[0:64] * (1 / O'[64]) columnwise.
                # broadcast the denominator row first so the reciprocal
                # runs on 64 partitions instead of 1 (a [1,512] DVE op is
                # ~4us; this chain is ~3us and mostly off-DVE).
                drow = small.tile([1, 512], F32, tag="drow")
                nc.vector.tensor_copy(drow[:], po[D:D + 1, :])
                rb = small.tile([D, 512], F32, tag="rb")
                nc.gpsimd.partition_broadcast(rb[:], drow[:], channels=D)
                rbr = small.tile([D, 512], F32, tag="rbr")
                nc.vector.reciprocal(rbr[:], rb[:])
                nc.vector.tensor_mul(
                    UnT[hs, qb * 512:(qb + 1) * 512], po[0:D, :], rbr[:]
                )
            # out-proj, software-pipelined one q block behind so the PE
            # never waits on the normalize chain
            if qb > 0:
                _outproj(nc, ps_mm, opool, UnT, wo_sb, out, qb - 1)
        _outproj(nc, ps_mm, opool, UnT, wo_sb, out, NT - 1)


def _outproj(nc, ps_mm, opool, UnT, wo_sb, out, qb):
    for tt in range(qb * 4, (qb + 1) * 4):
        osb2 = opool.tile([128, E], F32, tag="out")
        for half in range(2):
            psc = ps_mm.tile([128, 512], F32, tag="mm")
            nc.tensor.matmul(
                psc[:],
                UnT[:, tt * 128:(tt + 1) * 128],
                wo_sb[:, half * 512:(half + 1) * 512],
                start=True, stop=True,
            )
            nc.vector.tensor_copy(
                osb2[:, half * 512:(half + 1) * 512], psc[:]
            )
        nc.sync.dma_start(out[tt * 128:(tt + 1) * 128, :], osb2[:])


_NC_CACHE = None


def _get_nc():
    global _NC_CACHE
    if _NC_CACHE is None:
        _NC_CACHE = _build_kernel()
    return _NC_CACHE


def _make_in_maps(x, w_qkv, b_qkv, w_out):
    x2 = np.asarray(x, dtype=np.float32).reshape(T, E)
    xT = np.ascontiguousarray(x2.T).astype(NPBF16)
    w_qkv = np.asarray(w_qkv, dtype=np.float32)
    b_qkv = np.asarray(b_qkv, dtype=np.float32)
    w_out = np.asarray(w_out, dtype=np.float32)
    in_maps = []
    for c in range(NCORES):
        s = slice(c * HD, (c + 1) * HD)
        in_maps.append({
            "xT": xT,
            "wq": np.ascontiguousarray(
                w_qkv[:, 0 * E + c * HD:0 * E + (c + 1) * HD]).astype(NPBF16),
            "wk": np.ascontiguousarray(
                w_qkv[:, 1 * E + c * HD:1 * E + (c + 1) * HD]).astype(NPBF16),
            "wv": np.ascontiguousarray(
                w_qkv[:, 2 * E + c * HD:2 * E + (c + 1) * HD]).astype(NPBF16),
            "bqkv": np.ascontiguousarray(
                np.stack([
                    b_qkv[0 * E + c * HD:0 * E + (c + 1) * HD],
                    b_qkv[1 * E + c * HD:1 * E + (c + 1) * HD],
                    b_qkv[2 * E + c * HD:2 * E + (c + 1) * HD],
                ]).reshape(3, HD, 1)
            ),
            "wo": np.ascontiguousarray(w_out[s, :]).astype(NPBF16),
        })
    return in_maps


def run_sharded(x, w_qkv, b_qkv, w_out, b_out, trace=False):
    """Run the SPMD kernel; returns (full_output, BassKernelResults)."""
    nc = _get_nc()
    in_maps = _make_in_maps(x, w_qkv, b_qkv, w_out)
    res = run_bass_kernel_spmd(
        nc, in_maps, core_ids=list(range(NCORES)), trace=trace
    )
    acc = np.zeros((T, E), dtype=np.float32)
    for c in range(NCORES):
        acc += res.results[c]["out"]
    acc += np.asarray(b_out, dtype=np.float32)[None, :]
    return acc.reshape(1, T, E), res


def kernel(x, w_qkv, b_qkv, w_out, b_out):
    out, _ = run_sharded(x, w_qkv, b_qkv, w_out, b_out, trace=False)
    return out
